# revision 5
# baseline (speedup 1.0000x reference)
"""Trainium2 Bass kernel for per-sample-LoRA causal self-attention (non-causal SDPA).

Sharding: 8 cores = (batch b in 0..3) x (channel-half in 0..1).
Each core computes q/k/v for its 1024 output channels (8 heads) of sample b,
runs attention for those heads, and produces a partial output projection
(contraction over its half of the y channels). Host sums the two partials
per sample and transposes back.

All matmuls run as float32r (TF32-like, full PE rate, ~1e-4 rel accuracy).
"""

import os
import sys

sys.path.insert(0, "/opt/trn_rl_repo")

import numpy as np

import concourse.bass as bass  # noqa: F401
import concourse.mybir as mybir
import concourse.tile as tile
from concourse import bacc, bass_utils

F32 = mybir.dt.float32
F32R = mybir.dt.float32r
AF = mybir.ActivationFunctionType

B, T, C = 4, 1024, 2048
H, D, R = 16, 128, 16
HALF = C // 2          # output channels per core
HH = HALF // D         # heads per core = 8
CT = C // 128          # contraction tiles over C = 16
IT = HALF // 128       # contraction tiles over half = 8
CH = 512               # t/free chunk
NCH = T // CH          # = 2
PTP = 4                # s_tiles per pT part
SCALE = 1.0 / float(np.sqrt(D))
ROPE_BASE = 10000.0

_compiled = {}
last_result = None     # BassKernelResults of the most recent run (for test harness)


def _build_nc():
    nc = bacc.Bacc("TRN2", target_bir_lowering=False, debug=False, num_devices=8)

    xT = nc.dram_tensor("xT", [C, T], F32R, kind="ExternalInput").ap()
    WqT = nc.dram_tensor("WqT", [C, HALF], F32R, kind="ExternalInput").ap()
    WkT = nc.dram_tensor("WkT", [C, HALF], F32R, kind="ExternalInput").ap()
    WvT = nc.dram_tensor("WvT", [C, HALF], F32R, kind="ExternalInput").ap()
    WoT = nc.dram_tensor("WoT", [HALF, C], F32R, kind="ExternalInput").ap()
    dAT = nc.dram_tensor("dAT", [C, 80], F32R, kind="ExternalInput").ap()
    dBp = nc.dram_tensor("dBp", [80, HALF], F32R, kind="ExternalInput").ap()
    doAT = nc.dram_tensor("doAT", [HALF, R], F32R, kind="ExternalInput").ap()
    doB = nc.dram_tensor("doB", [R, C], F32R, kind="ExternalInput").ap()
    cosT = nc.dram_tensor("cosT", [D, T], F32, kind="ExternalInput").ap()
    sinTs = nc.dram_tensor("sinTs", [D, T], F32, kind="ExternalInput").ap()
    outT = nc.dram_tensor("outT", [C, T], F32, kind="ExternalOutput").ap()
    y_spill = nc.dram_tensor("y_spill", [HH, D, T], F32R).ap()

    with tile.TileContext(nc) as tc:
        with tc.tile_pool(name="tabs", bufs=1) as tabs, \
             tc.tile_pool(name="ps_acc", bufs=3, space="PSUM") as ps_acc, \
             tc.tile_pool(name="ps_s", bufs=2, space="PSUM") as ps_s, \
             tc.tile_pool(name="ps_y", bufs=1, space="PSUM") as ps_y, \
             tc.tile_pool(name="ps_l", bufs=1, space="PSUM") as ps_l, \
             tc.tile_pool(name="ps_b", bufs=1, space="PSUM") as ps_b:

            # ---------------- resident tables ----------------
            cos_sb = tabs.tile([D, T], F32)
            sin_sb = tabs.tile([D, T], F32)
            nc.sync.dma_start(cos_sb[:], cosT[:])
            nc.sync.dma_start(sin_sb[:], sinTs[:])
            const_f = tabs.tile([128, 129], F32)
            nc.gpsimd.memset(const_f[:], 1.0)
            const_r = tabs.tile([128, 129], F32R)
            nc.vector.tensor_copy(const_r[:], const_f[:])
            ones128 = const_r[:, 0:1]
            ones1 = const_r[0:1, 1:129]

            dB_sb = tabs.tile([80, HALF], F32R)
            nc.sync.dma_start(dB_sb[:], dBp[:])
            u_sb = tabs.tile([80, T], F32R)
            v_sb = tabs.tile([128, IT, HALF], F32R)   # [t_in_tile, t_tile, vo]

            # ============ phase 1: u, v, per-head qk+attention ============
            with tc.tile_pool(name="xpool", bufs=1) as xpool:
                x_sb = xpool.tile([128, CT, T], F32R)
                nc.sync.dma_start(x_sb[:], xT.rearrange("(ct p) t -> p ct t", p=128))

                # ---- u = [dqA;dkA;dvA] @ x^T  (rows padded to 0/32/64) ----
                with tc.tile_pool(name="dap", bufs=1) as dap:
                    dAT_sb = dap.tile([128, CT, 80], F32R)
                    nc.sync.dma_start(dAT_sb[:], dAT.rearrange("(ct p) r -> p ct r", p=128))
                    for ci in range(NCH):
                        ps_u = ps_l.tile([80, CH], F32, tag="l")
                        for ct in range(CT):
                            nc.tensor.matmul(ps_u[:], dAT_sb[:, ct, :],
                                             x_sb[:, ct, ci * CH:(ci + 1) * CH],
                                             start=(ct == 0), stop=(ct == CT - 1))
                        nc.scalar.activation(u_sb[:, ci * CH:(ci + 1) * CH], ps_u[:], AF.Copy)

                # ---- P1-v : v natural [t, vo] ----
                with tc.tile_pool(name="wv", bufs=2) as wvp:
                    for ci in range(2):                # vo chunk of 512
                        wv_half = wvp.tile([128, CT, CH], F32R, tag="wv")
                        nc.sync.dma_start(
                            wv_half[:],
                            WvT.rearrange("(ct p) o -> p ct o", p=128)[:, :, ci * CH:(ci + 1) * CH])
                        for tt in range(IT):
                            ps = ps_acc.tile([128, CH], F32, tag="acc")
                            for ct in range(CT):
                                nc.tensor.matmul(ps[:], x_sb[:, ct, tt * 128:(tt + 1) * 128],
                                                 wv_half[:, ct, :],
                                                 start=(ct == 0), stop=False)
                            nc.tensor.matmul(ps[:], u_sb[64:80, tt * 128:(tt + 1) * 128],
                                             dB_sb[64:80, ci * CH:(ci + 1) * CH],
                                             start=False, stop=True)
                            nc.scalar.activation(v_sb[:, tt, ci * CH:(ci + 1) * CH], ps[:],
                                                 AF.Copy)

                # ---- per-head: P1-qk + RoPE + attention ----
                with tc.tile_pool(name="wqk", bufs=6) as wqkp, \
                     tc.tile_pool(name="rope", bufs=2) as rope, \
                     tc.tile_pool(name="qk", bufs=2) as qkp, \
                     tc.tile_pool(name="ptp", bufs=3) as ptp, \
                     tc.tile_pool(name="att", bufs=1) as att:
                    for h in range(HH):
                        rots = []
                        for pi, (wT, urow) in enumerate(((WqT, 0), (WkT, 32))):
                            rot = qkp.tile([D, T], F32R, tag=f"rot{pi}")
                            for ci in range(NCH):
                                ps = ps_acc.tile([128, CH], F32, tag="acc")
                                for ct in range(CT):
                                    w = wqkp.tile([128, 128], F32R, tag="wqk")
                                    nc.sync.dma_start(
                                        w[:], wT[ct * 128:(ct + 1) * 128, h * D:(h + 1) * D])
                                    nc.tensor.matmul(ps[:], w[:],
                                                     x_sb[:, ct, ci * CH:(ci + 1) * CH],
                                                     start=(ct == 0), stop=False)
                                nc.tensor.matmul(ps[:], dB_sb[urow:urow + R, h * D:(h + 1) * D],
                                                 u_sb[urow:urow + R, ci * CH:(ci + 1) * CH],
                                                 start=False, stop=True)
                                # RoPE: PSUM -> SBUF copy, shift, mul, add -> rot (f32r)
                                q0 = rope.tile([D, CH], F32, tag="q0")
                                nc.scalar.activation(q0[:], ps[:], AF.Copy)
                                sh = rope.tile([D, CH], F32, tag="sh")
                                nc.sync.dma_start(sh[0:64, :], q0[64:128, :])
                                nc.sync.dma_start(sh[64:128, :], q0[0:64, :])
                                nc.vector.tensor_mul(sh[:], sh[:],
                                                     sin_sb[:, ci * CH:(ci + 1) * CH])
                                nc.vector.tensor_mul(q0[:], q0[:],
                                                     cos_sb[:, ci * CH:(ci + 1) * CH])
                                nc.vector.tensor_add(rot[:, ci * CH:(ci + 1) * CH],
                                                     q0[:], sh[:])
                            rots.append(rot)
                        qr, kr = rots

                        # A1 + exp -> pT in two half-parts of 4 s_tiles each
                        pts = []
                        for part in range(IT // PTP):
                            pT = ptp.tile([128, PTP, T], F32R, tag="pT")
                            for sp in range(PTP):
                                st = part * PTP + sp
                                for ci in range(NCH):
                                    ps = ps_s.tile([128, CH], F32, tag="s")
                                    nc.tensor.matmul(ps[:], kr[:, st * 128:(st + 1) * 128],
                                                     qr[:, ci * CH:(ci + 1) * CH],
                                                     start=True, stop=True)
                                    nc.scalar.activation(pT[:, sp, ci * CH:(ci + 1) * CH],
                                                         ps[:], AF.Exp, scale=SCALE)
                            pts.append(pT)

                        # l = column sums of p^T ; rb = broadcast reciprocal
                        rb = att.tile([128, T], F32, tag="rb")
                        l_sb = tabs.tile([1, T], F32R, tag="l_sb")
                        for ci in range(NCH):
                            l_ps = ps_l.tile([1, CH], F32, tag="l")
                            for st in range(IT):
                                nc.tensor.matmul(l_ps[:], ones128,
                                                 pts[st // PTP][:, st % PTP,
                                                                ci * CH:(ci + 1) * CH],
                                                 start=(st == 0), stop=(st == IT - 1))
                            nc.scalar.activation(l_sb[:, ci * CH:(ci + 1) * CH], l_ps[:],
                                                 AF.Copy)
                            lb = ps_b.tile([128, CH], F32, tag="lb")
                            nc.tensor.matmul(lb[:], ones1, l_sb[:, ci * CH:(ci + 1) * CH],
                                             start=True, stop=True)
                            nc.vector.reciprocal_approx_fast(
                                out=rb[:, ci * CH:(ci + 1) * CH], in_=lb[:])

                        # A2 + scale + spill
                        y_sc = att.tile([D, T], F32R, tag="y_sc")
                        for ci in range(NCH):
                            yp = ps_y.tile([D, CH], F32, tag="y")
                            for st in range(IT):
                                nc.tensor.matmul(yp[:], v_sb[:, st, h * D:(h + 1) * D],
                                                 pts[st // PTP][:, st % PTP,
                                                                ci * CH:(ci + 1) * CH],
                                                 start=(st == 0), stop=(st == IT - 1))
                            nc.vector.tensor_mul(y_sc[:, ci * CH:(ci + 1) * CH], yp[:],
                                                 rb[:, ci * CH:(ci + 1) * CH])
                        nc.sync.dma_start(y_spill[h], y_sc[:])

            # ============ phase 2: out^T = Wo-half contraction + LoRA ============
            with tc.tile_pool(name="p2", bufs=1) as p2p, \
                 tc.tile_pool(name="wo", bufs=2) as wop, \
                 tc.tile_pool(name="outp", bufs=3) as outp:
                y_sb = p2p.tile([128, HH, T], F32R)
                nc.sync.dma_start(y_sb[:], y_spill.rearrange("h p t -> p h t"))
                doB_sb = p2p.tile([R, C], F32R)
                nc.sync.dma_start(doB_sb[:], doB[:])
                doAT_sb = p2p.tile([128, IT, R], F32R)
                nc.sync.dma_start(doAT_sb[:], doAT.rearrange("(it p) r -> p it r", p=128))

                uo_sb = p2p.tile([R, T], F32R)
                for ci in range(NCH):
                    ps_uo = ps_l.tile([R, CH], F32, tag="l")
                    for it in range(IT):
                        nc.tensor.matmul(ps_uo[:], doAT_sb[:, it, :],
                                         y_sb[:, it, ci * CH:(ci + 1) * CH],
                                         start=(it == 0), stop=(it == IT - 1))
                    nc.scalar.activation(uo_sb[:, ci * CH:(ci + 1) * CH], ps_uo[:], AF.Copy)

                for ot in range(C // 128):
                    wo = wop.tile([128, IT, 128], F32R, tag="wo")
                    nc.sync.dma_start(
                        wo[:],
                        WoT.rearrange("(it p) o -> p it o", p=128)[:, :, ot * 128:(ot + 1) * 128])
                    for ci in range(NCH):
                        ps = ps_acc.tile([128, CH], F32, tag="acc")
                        for it in range(IT):
                            nc.tensor.matmul(ps[:], wo[:, it, :],
                                             y_sb[:, it, ci * CH:(ci + 1) * CH],
                                             start=(it == 0), stop=False)
                        nc.tensor.matmul(ps[:], doB_sb[:, ot * 128:(ot + 1) * 128],
                                         uo_sb[:, ci * CH:(ci + 1) * CH],
                                         start=False, stop=True)
                        o_sb = outp.tile([128, CH], F32, tag="o")
                        nc.scalar.activation(o_sb[:], ps[:], AF.Copy)
                        nc.sync.dma_start(outT[ot * 128:(ot + 1) * 128,
                                               ci * CH:(ci + 1) * CH], o_sb[:])

    nc.compile()
    return nc


def _rope_tables():
    inv = (1.0 / (ROPE_BASE ** (np.arange(0, D, 2, dtype=np.float32) / np.float32(D)))).astype(np.float32)
    t_ar = np.arange(T, dtype=np.float32)
    fr = t_ar[:, None] * inv[None, :]
    emb = np.concatenate([fr, fr], axis=1)          # [T, D]
    cos = np.cos(emb).astype(np.float32).T.copy()   # [D, T]
    sin = np.sin(emb).astype(np.float32).T.copy()
    sins = sin.copy()
    sins[:64, :] *= -1.0
    return np.ascontiguousarray(cos), np.ascontiguousarray(sins)


def kernel(x, qkvo_delta, Wq, Wk, Wv, Wo):
    global last_result
    x = np.asarray(x, dtype=np.float32)
    qkvo_delta = np.asarray(qkvo_delta, dtype=np.float32)
    Wq = np.asarray(Wq, dtype=np.float32)
    Wk = np.asarray(Wk, dtype=np.float32)
    Wv = np.asarray(Wv, dtype=np.float32)
    Wo = np.asarray(Wo, dtype=np.float32)

    if "nc" not in _compiled:
        _compiled["nc"] = _build_nc()
    nc = _compiled["nc"]

    cos, sins = _rope_tables()
    d = qkvo_delta.reshape(B, 8, R, C)
    dqA, dqB, dkA, dkB, dvA, dvB, doA, doB = (d[:, i] for i in range(8))

    in_maps = []
    for core in range(8):
        b, half = core // 2, core % 2
        sl = slice(half * HALF, (half + 1) * HALF)
        dAT = np.zeros((C, 80), dtype=np.float32)
        dAT[:, 0:16] = dqA[b].T
        dAT[:, 32:48] = dkA[b].T
        dAT[:, 64:80] = dvA[b].T
        dBp = np.zeros((80, HALF), dtype=np.float32)
        dBp[0:16] = dqB[b][:, sl]
        dBp[32:48] = dkB[b][:, sl]
        dBp[64:80] = dvB[b][:, sl]
        in_maps.append({
            "xT": np.ascontiguousarray(x[b].T),
            "WqT": np.ascontiguousarray(Wq[sl, :].T),
            "WkT": np.ascontiguousarray(Wk[sl, :].T),
            "WvT": np.ascontiguousarray(Wv[sl, :].T),
            "WoT": np.ascontiguousarray(Wo[:, sl].T),
            "dAT": dAT,
            "dBp": dBp,
            "doAT": np.ascontiguousarray(doA[b][:, sl].T),
            "doB": np.ascontiguousarray(doB[b]),
            "cosT": cos,
            "sinTs": sins,
        })

    trace = bool(int(os.environ.get("KERNEL_TRACE", "0")))
    res = bass_utils.run_bass_kernel_spmd(
        nc, in_maps, core_ids=list(range(8)), trace=trace)
    last_result = res

    out = np.empty((B, T, C), dtype=np.float32)
    for b in range(B):
        acc = res.results[2 * b]["outT"].astype(np.float32) + \
            res.results[2 * b + 1]["outT"].astype(np.float32)
        out[b] = acc.T
    return out


# revision 6
# speedup vs baseline: 1.2249x; 1.2249x over previous
"""Trainium2 Bass kernel for per-sample-LoRA causal self-attention (non-causal SDPA).

Sharding: 8 cores = (batch b in 0..3) x (channel-half in 0..1).
Each core computes q/k/v for its 1024 output channels (8 heads) of sample b,
runs attention for those heads, and produces a partial output projection
(contraction over its half of the y channels). Host sums the two partials
per sample and transposes back.

All matmuls run as float32r (TF32-like, full PE rate, ~1e-4 rel accuracy).
"""

import os
import sys

sys.path.insert(0, "/opt/trn_rl_repo")

import numpy as np

import concourse.bass as bass  # noqa: F401
import concourse.mybir as mybir
import concourse.tile as tile
from concourse import bacc, bass_utils

F32 = mybir.dt.float32
F32R = mybir.dt.float32r
AF = mybir.ActivationFunctionType

B, T, C = 4, 1024, 2048
H, D, R = 16, 128, 16
HALF = C // 2          # output channels per core
HH = HALF // D         # heads per core = 8
CT = C // 128          # contraction tiles over C = 16
IT = HALF // 128       # contraction tiles over half = 8
CH = 512               # t/free chunk
NCH = T // CH          # = 2
PTP = 2                # s_tiles per pT part
SCALE = 1.0 / float(np.sqrt(D))
ROPE_BASE = 10000.0

_compiled = {}
last_result = None     # BassKernelResults of the most recent run (for test harness)


def _build_nc():
    nc = bacc.Bacc("TRN2", target_bir_lowering=False, debug=False, num_devices=8)

    xT = nc.dram_tensor("xT", [C, T], F32R, kind="ExternalInput").ap()
    WqT = nc.dram_tensor("WqT", [C, HALF], F32R, kind="ExternalInput").ap()
    WkT = nc.dram_tensor("WkT", [C, HALF], F32R, kind="ExternalInput").ap()
    WvT = nc.dram_tensor("WvT", [C, HALF], F32R, kind="ExternalInput").ap()
    WoT = nc.dram_tensor("WoT", [HALF, C], F32R, kind="ExternalInput").ap()
    dAT = nc.dram_tensor("dAT", [C, 80], F32R, kind="ExternalInput").ap()
    dBp = nc.dram_tensor("dBp", [80, HALF], F32R, kind="ExternalInput").ap()
    doAT = nc.dram_tensor("doAT", [HALF, R], F32R, kind="ExternalInput").ap()
    doB = nc.dram_tensor("doB", [R, C], F32R, kind="ExternalInput").ap()
    cosT = nc.dram_tensor("cosT", [D, T], F32, kind="ExternalInput").ap()
    sinTs = nc.dram_tensor("sinTs", [D, T], F32, kind="ExternalInput").ap()
    outT = nc.dram_tensor("outT", [C, T], F32, kind="ExternalOutput").ap()
    y_spill = nc.dram_tensor("y_spill", [HH, D, T], F32R).ap()

    with tile.TileContext(nc) as tc:
        with tc.tile_pool(name="tabs", bufs=1) as tabs, \
             tc.tile_pool(name="ps_acc", bufs=3, space="PSUM") as ps_acc, \
             tc.tile_pool(name="ps_s", bufs=2, space="PSUM") as ps_s, \
             tc.tile_pool(name="ps_y", bufs=1, space="PSUM") as ps_y, \
             tc.tile_pool(name="ps_l", bufs=1, space="PSUM") as ps_l, \
             tc.tile_pool(name="ps_b", bufs=1, space="PSUM") as ps_b:

            # ---------------- resident tables ----------------
            cos_sb = tabs.tile([D, T], F32)
            sin_sb = tabs.tile([D, T], F32)
            nc.sync.dma_start(cos_sb[:], cosT[:])
            nc.sync.dma_start(sin_sb[:], sinTs[:])
            const_f = tabs.tile([128, 129], F32)
            nc.gpsimd.memset(const_f[:], 1.0)
            const_r = tabs.tile([128, 129], F32R)
            nc.vector.tensor_copy(const_r[:], const_f[:])
            ones128 = const_r[:, 0:1]
            ones1 = const_r[0:1, 1:129]

            dB_sb = tabs.tile([80, HALF], F32R)
            nc.sync.dma_start(dB_sb[:], dBp[:])
            u_sb = tabs.tile([80, T], F32R)
            v_sb = tabs.tile([128, IT, HALF], F32R)   # [t_in_tile, t_tile, vo]

            # ============ phase 1: u, v, per-head qk+attention ============
            with tc.tile_pool(name="xpool", bufs=1) as xpool:
                x_sb = xpool.tile([128, CT, T], F32R)
                xr = xT.rearrange("(ct p) t -> p ct t", p=128)
                for xg in range(8):
                    nc.sync.dma_start(x_sb[:, 2 * xg:2 * xg + 2, :], xr[:, 2 * xg:2 * xg + 2, :])

                # ---- u = [dqA;dkA;dvA] @ x^T  (rows padded to 0/32/64) ----
                with tc.tile_pool(name="dap", bufs=1) as dap:
                    dAT_sb = dap.tile([128, CT, 80], F32R)
                    nc.sync.dma_start(dAT_sb[:], dAT.rearrange("(ct p) r -> p ct r", p=128))
                    for ci in range(NCH):
                        ps_u = ps_l.tile([80, CH], F32, tag="l")
                        for ct in range(CT):
                            nc.tensor.matmul(ps_u[:], dAT_sb[:, ct, :],
                                             x_sb[:, ct, ci * CH:(ci + 1) * CH],
                                             start=(ct == 0), stop=(ct == CT - 1))
                        nc.scalar.activation(u_sb[:, ci * CH:(ci + 1) * CH], ps_u[:], AF.Copy)

                # ---- P1-v : v natural [t, vo] ----
                with tc.tile_pool(name="wv", bufs=2) as wvp:
                    for ci in range(2):                # vo chunk of 512
                        wv_half = wvp.tile([128, CT, CH], F32R, tag="wv")
                        nc.sync.dma_start(
                            wv_half[:],
                            WvT.rearrange("(ct p) o -> p ct o", p=128)[:, :, ci * CH:(ci + 1) * CH])
                        for tt in range(IT):
                            ps = ps_acc.tile([128, CH], F32, tag="acc")
                            for ct in range(CT):
                                nc.tensor.matmul(ps[:], x_sb[:, ct, tt * 128:(tt + 1) * 128],
                                                 wv_half[:, ct, :],
                                                 start=(ct == 0), stop=False)
                            nc.tensor.matmul(ps[:], u_sb[64:80, tt * 128:(tt + 1) * 128],
                                             dB_sb[64:80, ci * CH:(ci + 1) * CH],
                                             start=False, stop=True)
                            nc.scalar.activation(v_sb[:, tt, ci * CH:(ci + 1) * CH], ps[:],
                                                 AF.Copy)

                # ---- per-head: P1-qk + RoPE + attention ----
                with tc.tile_pool(name="wqk", bufs=4) as wqkp, \
                     tc.tile_pool(name="rope", bufs=2) as rope, \
                     tc.tile_pool(name="qk", bufs=3) as qkp, \
                     tc.tile_pool(name="ptp", bufs=5) as ptp, \
                     tc.tile_pool(name="att", bufs=1) as att:
                    for h in range(HH):
                        rots = []
                        for pi, (wT, urow) in enumerate(((WqT, 0), (WkT, 32))):
                            rot = qkp.tile([D, T], F32R, tag="rot")
                            slabs = []
                            for wh in range(2):
                                ws = wqkp.tile([128, CT // 2, 128], F32R, tag="wqk")
                                nc.sync.dma_start(
                                    ws[:],
                                    wT.rearrange("(ct p) o -> p ct o", p=128)[
                                        :, wh * (CT // 2):(wh + 1) * (CT // 2),
                                        h * D:(h + 1) * D])
                                slabs.append(ws)
                            for ci in range(NCH):
                                ps = ps_acc.tile([128, CH], F32, tag="acc")
                                for ct in range(CT):
                                    nc.tensor.matmul(ps[:],
                                                     slabs[ct // (CT // 2)][:, ct % (CT // 2), :],
                                                     x_sb[:, ct, ci * CH:(ci + 1) * CH],
                                                     start=(ct == 0), stop=False)
                                nc.tensor.matmul(ps[:], dB_sb[urow:urow + R, h * D:(h + 1) * D],
                                                 u_sb[urow:urow + R, ci * CH:(ci + 1) * CH],
                                                 start=False, stop=True)
                                # RoPE: PSUM -> SBUF copy, shift, mul, add -> rot (f32r)
                                q0 = rope.tile([D, CH], F32, tag="q0")
                                nc.scalar.activation(q0[:], ps[:], AF.Copy)
                                sh = rope.tile([D, CH], F32, tag="sh")
                                nc.sync.dma_start(sh[0:64, :], q0[64:128, :])
                                nc.sync.dma_start(sh[64:128, :], q0[0:64, :])
                                nc.vector.tensor_mul(sh[:], sh[:],
                                                     sin_sb[:, ci * CH:(ci + 1) * CH])
                                nc.vector.tensor_mul(q0[:], q0[:],
                                                     cos_sb[:, ci * CH:(ci + 1) * CH])
                                nc.vector.tensor_add(rot[:, ci * CH:(ci + 1) * CH],
                                                     q0[:], sh[:])
                            rots.append(rot)
                        qr, kr = rots

                        # A1 + exp -> pT in parts of PTP s_tiles each
                        pts = []
                        for part in range(IT // PTP):
                            pT = ptp.tile([128, PTP, T], F32R, tag="pT")
                            for ci in range(NCH):
                                for sp in range(PTP):
                                    st = part * PTP + sp
                                    ps = ps_s.tile([128, CH], F32, tag="s")
                                    nc.tensor.matmul(ps[:], kr[:, st * 128:(st + 1) * 128],
                                                     qr[:, ci * CH:(ci + 1) * CH],
                                                     start=True, stop=True)
                                    nc.scalar.activation(pT[:, sp, ci * CH:(ci + 1) * CH],
                                                         ps[:], AF.Exp, scale=SCALE)
                            pts.append(pT)

                        # l = column sums of p^T ; rb = broadcast reciprocal
                        rb = att.tile([128, T], F32, tag="rb")
                        l_sb = tabs.tile([1, T], F32R, tag="l_sb")
                        for ci in range(NCH):
                            l_ps = ps_l.tile([1, CH], F32, tag="l")
                            for st in range(IT):
                                nc.tensor.matmul(l_ps[:], ones128,
                                                 pts[st // PTP][:, st % PTP,
                                                                ci * CH:(ci + 1) * CH],
                                                 start=(st == 0), stop=(st == IT - 1))
                            nc.scalar.activation(l_sb[:, ci * CH:(ci + 1) * CH], l_ps[:],
                                                 AF.Copy)
                            lb = ps_b.tile([128, CH], F32, tag="lb")
                            nc.tensor.matmul(lb[:], ones1, l_sb[:, ci * CH:(ci + 1) * CH],
                                             start=True, stop=True)
                            nc.vector.reciprocal_approx_fast(
                                out=rb[:, ci * CH:(ci + 1) * CH], in_=lb[:])

                        # A2 + scale + spill
                        y_sc = att.tile([D, T], F32R, tag="y_sc")
                        for ci in range(NCH):
                            yp = ps_y.tile([D, CH], F32, tag="y")
                            for st in range(IT):
                                nc.tensor.matmul(yp[:], v_sb[:, st, h * D:(h + 1) * D],
                                                 pts[st // PTP][:, st % PTP,
                                                                ci * CH:(ci + 1) * CH],
                                                 start=(st == 0), stop=(st == IT - 1))
                            nc.vector.tensor_mul(y_sc[:, ci * CH:(ci + 1) * CH], yp[:],
                                                 rb[:, ci * CH:(ci + 1) * CH])
                        nc.sync.dma_start(y_spill[h], y_sc[:])

            # ============ phase 2: out^T = Wo-half contraction + LoRA ============
            with tc.tile_pool(name="p2", bufs=1) as p2p, \
                 tc.tile_pool(name="wo", bufs=2) as wop, \
                 tc.tile_pool(name="outp", bufs=3) as outp:
                y_sb = p2p.tile([128, HH, T], F32R)
                nc.sync.dma_start(y_sb[:], y_spill.rearrange("h p t -> p h t"))
                doB_sb = p2p.tile([R, C], F32R)
                nc.sync.dma_start(doB_sb[:], doB[:])
                doAT_sb = p2p.tile([128, IT, R], F32R)
                nc.sync.dma_start(doAT_sb[:], doAT.rearrange("(it p) r -> p it r", p=128))

                uo_sb = p2p.tile([R, T], F32R)
                for ci in range(NCH):
                    ps_uo = ps_l.tile([R, CH], F32, tag="l")
                    for it in range(IT):
                        nc.tensor.matmul(ps_uo[:], doAT_sb[:, it, :],
                                         y_sb[:, it, ci * CH:(ci + 1) * CH],
                                         start=(it == 0), stop=(it == IT - 1))
                    nc.scalar.activation(uo_sb[:, ci * CH:(ci + 1) * CH], ps_uo[:], AF.Copy)

                for ot in range(C // 128):
                    wo = wop.tile([128, IT, 128], F32R, tag="wo")
                    nc.sync.dma_start(
                        wo[:],
                        WoT.rearrange("(it p) o -> p it o", p=128)[:, :, ot * 128:(ot + 1) * 128])
                    for ci in range(NCH):
                        ps = ps_acc.tile([128, CH], F32, tag="acc")
                        for it in range(IT):
                            nc.tensor.matmul(ps[:], wo[:, it, :],
                                             y_sb[:, it, ci * CH:(ci + 1) * CH],
                                             start=(it == 0), stop=False)
                        nc.tensor.matmul(ps[:], doB_sb[:, ot * 128:(ot + 1) * 128],
                                         uo_sb[:, ci * CH:(ci + 1) * CH],
                                         start=False, stop=True)
                        o_sb = outp.tile([128, CH], F32, tag="o")
                        nc.scalar.activation(o_sb[:], ps[:], AF.Copy)
                        nc.sync.dma_start(outT[ot * 128:(ot + 1) * 128,
                                               ci * CH:(ci + 1) * CH], o_sb[:])

    nc.compile()
    return nc


def _rope_tables():
    inv = (1.0 / (ROPE_BASE ** (np.arange(0, D, 2, dtype=np.float32) / np.float32(D)))).astype(np.float32)
    t_ar = np.arange(T, dtype=np.float32)
    fr = t_ar[:, None] * inv[None, :]
    emb = np.concatenate([fr, fr], axis=1)          # [T, D]
    cos = np.cos(emb).astype(np.float32).T.copy()   # [D, T]
    sin = np.sin(emb).astype(np.float32).T.copy()
    sins = sin.copy()
    sins[:64, :] *= -1.0
    return np.ascontiguousarray(cos), np.ascontiguousarray(sins)


def kernel(x, qkvo_delta, Wq, Wk, Wv, Wo):
    global last_result
    x = np.asarray(x, dtype=np.float32)
    qkvo_delta = np.asarray(qkvo_delta, dtype=np.float32)
    Wq = np.asarray(Wq, dtype=np.float32)
    Wk = np.asarray(Wk, dtype=np.float32)
    Wv = np.asarray(Wv, dtype=np.float32)
    Wo = np.asarray(Wo, dtype=np.float32)

    if "nc" not in _compiled:
        _compiled["nc"] = _build_nc()
    nc = _compiled["nc"]

    cos, sins = _rope_tables()
    d = qkvo_delta.reshape(B, 8, R, C)
    dqA, dqB, dkA, dkB, dvA, dvB, doA, doB = (d[:, i] for i in range(8))

    in_maps = []
    for core in range(8):
        b, half = core // 2, core % 2
        sl = slice(half * HALF, (half + 1) * HALF)
        dAT = np.zeros((C, 80), dtype=np.float32)
        dAT[:, 0:16] = dqA[b].T
        dAT[:, 32:48] = dkA[b].T
        dAT[:, 64:80] = dvA[b].T
        dBp = np.zeros((80, HALF), dtype=np.float32)
        dBp[0:16] = dqB[b][:, sl]
        dBp[32:48] = dkB[b][:, sl]
        dBp[64:80] = dvB[b][:, sl]
        in_maps.append({
            "xT": np.ascontiguousarray(x[b].T),
            "WqT": np.ascontiguousarray(Wq[sl, :].T),
            "WkT": np.ascontiguousarray(Wk[sl, :].T),
            "WvT": np.ascontiguousarray(Wv[sl, :].T),
            "WoT": np.ascontiguousarray(Wo[:, sl].T),
            "dAT": dAT,
            "dBp": dBp,
            "doAT": np.ascontiguousarray(doA[b][:, sl].T),
            "doB": np.ascontiguousarray(doB[b]),
            "cosT": cos,
            "sinTs": sins,
        })

    trace = bool(int(os.environ.get("KERNEL_TRACE", "0")))
    res = bass_utils.run_bass_kernel_spmd(
        nc, in_maps, core_ids=list(range(8)), trace=trace)
    last_result = res

    out = np.empty((B, T, C), dtype=np.float32)
    for b in range(B):
        acc = res.results[2 * b]["outT"].astype(np.float32) + \
            res.results[2 * b + 1]["outT"].astype(np.float32)
        out[b] = acc.T
    return out
